# revision 4
# baseline (speedup 1.0000x reference)
"""Bass/Tile kernel for masked dot-product attention on 8 Trainium2 cores.

Problem: queries/keys/values [128, 1024, 64] fp32, valid_lens [128] int32.
  out[b] = softmax(mask(Q K^T / 8, valid_lens[b])) @ V

Strategy:
  * Shard the 128 batch*heads across 8 cores, 16 head-slots per core.
    Heads are sorted by valid_len (descending) and dealt round-robin so
    every core gets the same per-slot chunk count -> one SPMD program.
  * Per head, only ceil(valid_len/128) key chunks contribute (the rest are
    fully masked -> softmax weight exactly 0), so the program is
    specialized to skip them (~45% of the work for uniform valid_lens).
  * Layout: compute S^T = K Q^T chunkwise on the PE ([128 k x 1024 q]),
    so the PV matmul can consume P^T directly as the moving operand.
    Masking + 1/sqrt(d) scaling + exp run as a single ScalarE activation
    (bias = per-partition mask column of 0 / -1e6; no max subtraction is
    needed: scores are bounded and exp(-1e6) underflows to exactly 0,
    matching the fp32 reference).
  * Softmax denominators come free: a ones-column is appended to V, so
    the PV accumulation produces [O^T ; sum_k P^T] in one pass.
    Normalization happens after a final PE transpose, where the
    denominator is a per-partition scalar.
  * Heads with valid_len == 0 (reference: uniform attention) are fixed up
    on the host with the exact reference semantics (mean of V).
"""

import math
from contextlib import ExitStack

import numpy as np

import concourse.bass as bass  # noqa: F401  (engine namespaces live on the nc)
import concourse.mybir as mybir
import concourse.tile as tile
from concourse import bacc
from concourse.bass_utils import run_bass_kernel_spmd
from concourse.masks import make_identity

BH, L, D = 128, 1024, 64
NCORES = 8
SLOTS = BH // NCORES  # 16
CHUNK = 128
NCH = L // CHUNK  # 8
MASK_VALUE = -1000000.0
F32 = mybir.dt.float32
MM_DT = mybir.dt.float32r  # fast fp32 matmul mode (full rate at N>=512)

_program_cache: dict = {}


def _build_program(m_list):
    nc = bacc.Bacc("TRN2", target_bir_lowering=False, debug=False)
    q_d = nc.dram_tensor("q", [SLOTS, L, D], F32, kind="ExternalInput").ap()
    k_d = nc.dram_tensor("k", [SLOTS, L, D], F32, kind="ExternalInput").ap()
    v_d = nc.dram_tensor("v", [SLOTS, L, D], F32, kind="ExternalInput").ap()
    mb_d = nc.dram_tensor("mb", [CHUNK, SLOTS * NCH], F32, kind="ExternalInput").ap()
    o_d = nc.dram_tensor("o", [SLOTS, L, D], F32, kind="ExternalOutput").ap()

    Exp = mybir.ActivationFunctionType.Exp

    with tile.TileContext(nc) as tc, ExitStack() as ctx:
        const = ctx.enter_context(tc.tile_pool(name="const", bufs=1))
        ident = const.tile([128, 128], F32)
        make_identity(nc, ident)
        mb = const.tile([CHUNK, SLOTS * NCH], F32)
        nc.sync.dma_start(mb[:], mb_d[:])
        ones = const.tile([128, 1], F32)
        nc.gpsimd.memset(ones[:], 1.0)

        qnat_p = ctx.enter_context(tc.tile_pool(name="qnat", bufs=2))
        knat_p = ctx.enter_context(tc.tile_pool(name="knat", bufs=2))
        qt_p = ctx.enter_context(tc.tile_pool(name="qt", bufs=2))
        kt_p = ctx.enter_context(tc.tile_pool(name="kt", bufs=2))
        vnat_p = ctx.enter_context(tc.tile_pool(name="vnat", bufs=2))
        vp_p = ctx.enter_context(tc.tile_pool(name="vp", bufs=2))
        pt_p = ctx.enter_context(tc.tile_pool(name="pt", bufs=3))
        ot_p = ctx.enter_context(tc.tile_pool(name="ot", bufs=2))
        osb_p = ctx.enter_context(tc.tile_pool(name="osb", bufs=4))
        rec_p = ctx.enter_context(tc.tile_pool(name="rec", bufs=4))

        tp_ps = ctx.enter_context(tc.tile_pool(name="tp", bufs=1, space="PSUM"))
        s_ps = ctx.enter_context(tc.tile_pool(name="s", bufs=2, space="PSUM"))
        o_ps = ctx.enter_context(tc.tile_pool(name="ops", bufs=1, space="PSUM"))
        tt_ps = ctx.enter_context(tc.tile_pool(name="tt", bufs=1, space="PSUM"))

        for j in range(SLOTS):
            m = m_list[j]

            # Q: load all 8 row-chunks, PE-transpose to qt [64(d), 1024(q)].
            qnat = qnat_p.tile([128, NCH * D], F32)
            for c in range(NCH):
                nc.sync.dma_start(
                    qnat[:, c * D : (c + 1) * D], q_d[j, c * CHUNK : (c + 1) * CHUNK, :]
                )
            qt = qt_p.tile([64, L], MM_DT)
            for g in range(2):
                tp = tp_ps.tile([64, 512], F32)
                for cc in range(4):
                    c = 4 * g + cc
                    nc.tensor.transpose(
                        tp[:, cc * 128 : (cc + 1) * 128],
                        qnat[:, c * D : (c + 1) * D],
                        ident[:, :],
                    )
                nc.vector.tensor_copy(qt[:, g * 512 : (g + 1) * 512], tp[:])

            # K: only the first m chunks matter; transpose to kt [64, 128*m].
            knat = knat_p.tile([128, NCH * D], F32)
            for c in range(m):
                nc.sync.dma_start(
                    knat[:, c * D : (c + 1) * D], k_d[j, c * CHUNK : (c + 1) * CHUNK, :]
                )
            kt = kt_p.tile([64, L], MM_DT)
            for g in range(math.ceil(m / 4)):
                lo = 4 * g
                hi = min(m, lo + 4)
                tp = tp_ps.tile([64, 512], F32)
                for cc in range(hi - lo):
                    c = lo + cc
                    nc.tensor.transpose(
                        tp[:, cc * 128 : (cc + 1) * 128],
                        knat[:, c * D : (c + 1) * D],
                        ident[:, :],
                    )
                nc.vector.tensor_copy(
                    kt[:, lo * 128 : hi * 128], tp[:, 0 : (hi - lo) * 128]
                )

            # V chunks with an appended ones-column (softmax denominator).
            vnat = vnat_p.tile([128, NCH * D], F32)
            for c in range(m):
                nc.sync.dma_start(
                    vnat[:, c * D : (c + 1) * D], v_d[j, c * CHUNK : (c + 1) * CHUNK, :]
                )
            vp = vp_p.tile([128, NCH * (D + 1)], MM_DT)
            for c in range(m):
                base = c * (D + 1)
                nc.vector.tensor_copy(vp[:, base : base + D], vnat[:, c * D : (c + 1) * D])
                nc.vector.tensor_copy(vp[:, base + D : base + D + 1], ones[:])

            # Main loop over key chunks: S^T = K_c Q^T, P^T = exp(S^T/8+mask),
            # [O^T ; denom] += [V_c | 1]^T P^T.
            opsum = o_ps.tile([65, L], F32)
            for c in range(m):
                s = s_ps.tile([128, L], F32)
                lhsT = kt[:, c * 128 : (c + 1) * 128]
                for h in range(2):
                    nc.tensor.matmul(
                        s[:, h * 512 : (h + 1) * 512],
                        lhsT,
                        qt[:, h * 512 : (h + 1) * 512],
                        start=True,
                        stop=True,
                    )
                pt = pt_p.tile([128, L], MM_DT)
                col = j * NCH + c
                nc.scalar.activation(
                    pt[:], s[:], Exp, bias=mb[:, col : col + 1], scale=0.125
                )
                vl = vp[:, c * (D + 1) : (c + 1) * (D + 1)]
                for h in range(2):
                    nc.tensor.matmul(
                        opsum[:, h * 512 : (h + 1) * 512],
                        vl,
                        pt[:, h * 512 : (h + 1) * 512],
                        start=(c == 0),
                        stop=(c == m - 1),
                    )

            # Epilogue: transpose [O^T ; denom] back and normalize.
            ot = ot_p.tile([65, L], F32)
            nc.vector.tensor_copy(ot[:], opsum[:])
            for g in range(NCH):
                tt = tt_ps.tile([128, 65], F32)
                nc.tensor.transpose(
                    tt[:], ot[:, g * 128 : (g + 1) * 128], ident[0:65, 0:65]
                )
                rec = rec_p.tile([128, 1], F32)
                nc.vector.reciprocal(rec[:], tt[:, 64:65])
                osb = osb_p.tile([128, D], F32)
                nc.vector.tensor_scalar_mul(osb[:], tt[:, 0:64], rec[:])
                nc.sync.dma_start(o_d[j, g * CHUNK : (g + 1) * CHUNK, :], osb[:])

    nc.compile()
    return nc


def _plan(valid_lens):
    """Sort heads by valid_len desc, deal round-robin across cores.

    Returns (assign [NCORES, SLOTS] head indices, m_list [SLOTS] chunk counts).
    """
    order = np.argsort(-valid_lens, kind="stable")
    assign = order.reshape(SLOTS, NCORES).T  # [core, slot]
    m_list = []
    for j in range(SLOTS):
        vmax = int(valid_lens[assign[:, j]].max())
        m_list.append(min(NCH, max(1, math.ceil(vmax / CHUNK))))
    return assign, m_list


def _run(queries, keys, values, valid_lens, trace=False):
    queries = np.ascontiguousarray(np.asarray(queries, dtype=np.float32))
    keys = np.ascontiguousarray(np.asarray(keys, dtype=np.float32))
    values = np.ascontiguousarray(np.asarray(values, dtype=np.float32))
    valid_lens = np.asarray(valid_lens, dtype=np.int32)

    assign, m_list = _plan(valid_lens)

    key = tuple(m_list)
    nc = _program_cache.get(key)
    if nc is None:
        nc = _build_program(m_list)
        _program_cache[key] = nc

    kk = np.arange(L, dtype=np.int64)
    in_maps = []
    for i in range(NCORES):
        heads = assign[i]
        mask = np.where(
            kk[None, :] < valid_lens[heads][:, None], 0.0, MASK_VALUE
        ).astype(np.float32)  # [SLOTS, L]
        # mb[p, j*NCH+c] = mask for key index c*128+p of slot j.
        mb = np.transpose(mask.reshape(SLOTS, NCH, CHUNK), (2, 0, 1)).reshape(
            CHUNK, SLOTS * NCH
        )
        in_maps.append(
            {
                "q": queries[heads],
                "k": keys[heads],
                "v": values[heads],
                "mb": np.ascontiguousarray(mb),
            }
        )

    res = run_bass_kernel_spmd(nc, in_maps, list(range(NCORES)), trace=trace)

    out = np.empty((BH, L, D), dtype=np.float32)
    for i in range(NCORES):
        out[assign[i]] = res.results[i]["o"]

    # valid_len == 0: reference softmaxes an all-masked row -> uniform weights.
    for h in np.nonzero(valid_lens == 0)[0]:
        out[h] = values[h].mean(axis=0, keepdims=True)

    return out, res


def kernel(queries, keys, values, valid_lens):
    out, _ = _run(queries, keys, values, valid_lens)
    return out


# revision 6
# speedup vs baseline: 1.1438x; 1.1438x over previous
"""Bass/Tile kernel for masked dot-product attention on 8 Trainium2 cores.

Problem: queries/keys/values [128, 1024, 64] fp32, valid_lens [128] int32.
  out[b] = softmax(mask(Q K^T / 8, valid_lens[b])) @ V

Strategy:
  * Shard the 128 batch*heads across 8 cores, 16 head-slots per core.
    Heads are sorted by valid_len (descending) and dealt round-robin so
    every core gets the same per-slot chunk count -> one SPMD program.
  * Per head, only ceil(valid_len/128) key chunks contribute (the rest are
    fully masked -> softmax weight exactly 0), so the program is
    specialized to skip them (~45% of the work for uniform valid_lens).
  * Layout: compute S^T = K Q^T chunkwise on the PE ([128 k x 1024 q]),
    so the PV matmul can consume P^T directly as the moving operand.
    Masking + 1/sqrt(d) scaling + exp run as a single ScalarE activation
    (bias = per-partition mask column of 0 / -1e6; no max subtraction is
    needed: scores are bounded and exp(-1e6) underflows to exactly 0,
    matching the fp32 reference).
  * Softmax denominators come free: a ones-column is appended to V, so
    the PV accumulation produces [O^T ; sum_k P^T] in one pass.
    Normalization happens after a final PE transpose, where the
    denominator is a per-partition scalar.
  * Heads with valid_len == 0 (reference: uniform attention) are fixed up
    on the host with the exact reference semantics (mean of V).
"""

import math
from contextlib import ExitStack

import numpy as np

import concourse.bass as bass  # noqa: F401  (engine namespaces live on the nc)
import concourse.mybir as mybir
import concourse.tile as tile
from concourse import bacc
from concourse.bass_utils import run_bass_kernel_spmd
from concourse.masks import make_identity

BH, L, D = 128, 1024, 64
NCORES = 8
SLOTS = BH // NCORES  # 16
CHUNK = 128
NCH = L // CHUNK  # 8
MASK_VALUE = -1000000.0
F32 = mybir.dt.float32
MM_DT = mybir.dt.float32r  # fast fp32 matmul mode (full rate at N>=512)

_program_cache: dict = {}


def _build_program(m_list):
    nc = bacc.Bacc("TRN2", target_bir_lowering=False, debug=False)
    q_d = nc.dram_tensor("q", [SLOTS, L, D], F32, kind="ExternalInput").ap()
    k_d = nc.dram_tensor("k", [SLOTS, L, D], F32, kind="ExternalInput").ap()
    v_d = nc.dram_tensor("v", [SLOTS, L, D], F32, kind="ExternalInput").ap()
    mb_d = nc.dram_tensor("mb", [CHUNK, SLOTS * NCH], F32, kind="ExternalInput").ap()
    o_d = nc.dram_tensor("o", [SLOTS, L, D], F32, kind="ExternalOutput").ap()

    Exp = mybir.ActivationFunctionType.Exp

    with tile.TileContext(nc) as tc, ExitStack() as ctx:
        const = ctx.enter_context(tc.tile_pool(name="const", bufs=1))
        ident = const.tile([128, 128], F32)
        make_identity(nc, ident)
        mb = const.tile([CHUNK, SLOTS * NCH], F32)
        nc.sync.dma_start(mb[:], mb_d[:])
        ones = const.tile([128, 1], F32)
        nc.gpsimd.memset(ones[:], 1.0)

        qnat_p = ctx.enter_context(tc.tile_pool(name="qnat", bufs=2))
        knat_p = ctx.enter_context(tc.tile_pool(name="knat", bufs=2))
        qt_p = ctx.enter_context(tc.tile_pool(name="qt", bufs=2))
        kt_p = ctx.enter_context(tc.tile_pool(name="kt", bufs=2))
        vnat_p = ctx.enter_context(tc.tile_pool(name="vnat", bufs=2))
        vp_p = ctx.enter_context(tc.tile_pool(name="vp", bufs=2))
        pt_p = ctx.enter_context(tc.tile_pool(name="pt", bufs=3))
        ot_p = ctx.enter_context(tc.tile_pool(name="ot", bufs=2))
        osb_p = ctx.enter_context(tc.tile_pool(name="osb", bufs=4))
        rec_p = ctx.enter_context(tc.tile_pool(name="rec", bufs=4))

        tp_ps = ctx.enter_context(tc.tile_pool(name="tp", bufs=1, space="PSUM"))
        s_ps = ctx.enter_context(tc.tile_pool(name="s", bufs=2, space="PSUM"))
        o_ps = ctx.enter_context(tc.tile_pool(name="ops", bufs=1, space="PSUM"))
        tt_ps = ctx.enter_context(tc.tile_pool(name="tt", bufs=1, space="PSUM"))

        for j in range(SLOTS):
            m = m_list[j]

            # Strided single-DMA loads: DRAM [1024, 64] -> SBUF [128, c*64]
            # (partition = row-within-chunk, chunks side by side in free dim).
            qnat = qnat_p.tile([128, NCH * D], F32)
            nc.sync.dma_start(
                qnat[:].rearrange("p (c d) -> p c d", d=D),
                q_d[j].rearrange("(c p) d -> p c d", p=CHUNK),
            )
            knat = knat_p.tile([128, NCH * D], F32)
            nc.sync.dma_start(
                knat[:, 0 : m * D].rearrange("p (c d) -> p c d", d=D),
                k_d[j, 0 : m * CHUNK].rearrange("(c p) d -> p c d", p=CHUNK),
            )
            vnat = vnat_p.tile([128, NCH * D], F32)
            nc.sync.dma_start(
                vnat[:, 0 : m * D].rearrange("p (c d) -> p c d", d=D),
                v_d[j, 0 : m * CHUNK].rearrange("(c p) d -> p c d", p=CHUNK),
            )

            # PE-transpose Q to qt [64(d), 1024(q)] (one PSUM round + one copy).
            qt = qt_p.tile([64, L], MM_DT)
            tp = tp_ps.tile([64, L], F32)
            for c in range(NCH):
                nc.tensor.transpose(
                    tp[:, c * 128 : (c + 1) * 128],
                    qnat[:, c * D : (c + 1) * D],
                    ident[:, :],
                )
            nc.vector.tensor_copy(qt[:], tp[:])

            # Same for the m live K chunks.
            kt = kt_p.tile([64, L], MM_DT)
            tp = tp_ps.tile([64, L], F32)
            for c in range(m):
                nc.tensor.transpose(
                    tp[:, c * 128 : (c + 1) * 128],
                    knat[:, c * D : (c + 1) * D],
                    ident[:, :],
                )
            nc.vector.tensor_copy(kt[:, 0 : m * 128], tp[:, 0 : m * 128])

            # V chunks (cast to fp32r) with an appended ones-column.
            vp = vp_p.tile([128, NCH * (D + 1)], MM_DT)
            nc.vector.tensor_copy(
                vp[:].rearrange("p (c e) -> p c e", e=D + 1)[:, 0:m, 0:D],
                vnat[:, 0 : m * D].rearrange("p (c d) -> p c d", d=D),
            )
            for c in range(m):
                base = c * (D + 1)
                nc.vector.tensor_copy(vp[:, base + D : base + D + 1], ones[:])

            # Main loop over key chunks: S^T = K_c Q^T, P^T = exp(S^T/8+mask),
            # [O^T ; denom] += [V_c | 1]^T P^T.
            opsum = o_ps.tile([65, L], F32)
            for c in range(m):
                s = s_ps.tile([128, L], F32, tag="s")
                lhsT = kt[:, c * 128 : (c + 1) * 128]
                for h in range(2):
                    nc.tensor.matmul(
                        s[:, h * 512 : (h + 1) * 512],
                        lhsT,
                        qt[:, h * 512 : (h + 1) * 512],
                        start=True,
                        stop=True,
                    )
                pt = pt_p.tile([128, L], MM_DT)
                col = j * NCH + c
                nc.scalar.activation(
                    pt[:], s[:], Exp, bias=mb[:, col : col + 1], scale=0.125
                )
                vl = vp[:, c * (D + 1) : (c + 1) * (D + 1)]
                for h in range(2):
                    nc.tensor.matmul(
                        opsum[:, h * 512 : (h + 1) * 512],
                        vl,
                        pt[:, h * 512 : (h + 1) * 512],
                        start=(c == 0),
                        stop=(c == m - 1),
                    )

            # Epilogue: transpose [O^T ; denom] back, normalize, one store.
            ot = ot_p.tile([65, L], F32)
            nc.vector.tensor_copy(ot[:], opsum[:])
            osb = osb_p.tile([128, NCH * D], F32)
            for g in range(NCH):
                tt = s_ps.tile([128, 65], F32, tag="s")
                nc.tensor.transpose(
                    tt[:], ot[:, g * 128 : (g + 1) * 128], ident[0:65, 0:65]
                )
                rec = rec_p.tile([128, 1], F32)
                nc.vector.reciprocal(rec[:], tt[:, 64:65])
                nc.vector.tensor_scalar_mul(
                    osb[:, g * D : (g + 1) * D], tt[:, 0:64], rec[:]
                )
            nc.scalar.dma_start(
                o_d[j].rearrange("(g p) d -> p g d", p=CHUNK),
                osb[:].rearrange("p (g d) -> p g d", d=D),
            )

    nc.compile()
    return nc


def _plan(valid_lens):
    """Sort heads by valid_len desc, deal round-robin across cores.

    Returns (assign [NCORES, SLOTS] head indices, m_list [SLOTS] chunk counts).
    """
    order = np.argsort(-valid_lens, kind="stable")
    assign = order.reshape(SLOTS, NCORES).T  # [core, slot]
    m_list = []
    for j in range(SLOTS):
        vmax = int(valid_lens[assign[:, j]].max())
        m_list.append(min(NCH, max(1, math.ceil(vmax / CHUNK))))
    return assign, m_list


def _run(queries, keys, values, valid_lens, trace=False):
    queries = np.ascontiguousarray(np.asarray(queries, dtype=np.float32))
    keys = np.ascontiguousarray(np.asarray(keys, dtype=np.float32))
    values = np.ascontiguousarray(np.asarray(values, dtype=np.float32))
    valid_lens = np.asarray(valid_lens, dtype=np.int32)

    assign, m_list = _plan(valid_lens)

    key = tuple(m_list)
    nc = _program_cache.get(key)
    if nc is None:
        nc = _build_program(m_list)
        _program_cache[key] = nc

    kk = np.arange(L, dtype=np.int64)
    in_maps = []
    for i in range(NCORES):
        heads = assign[i]
        mask = np.where(
            kk[None, :] < valid_lens[heads][:, None], 0.0, MASK_VALUE
        ).astype(np.float32)  # [SLOTS, L]
        # mb[p, j*NCH+c] = mask for key index c*128+p of slot j.
        mb = np.transpose(mask.reshape(SLOTS, NCH, CHUNK), (2, 0, 1)).reshape(
            CHUNK, SLOTS * NCH
        )
        in_maps.append(
            {
                "q": queries[heads],
                "k": keys[heads],
                "v": values[heads],
                "mb": np.ascontiguousarray(mb),
            }
        )

    res = run_bass_kernel_spmd(nc, in_maps, list(range(NCORES)), trace=trace)

    out = np.empty((BH, L, D), dtype=np.float32)
    for i in range(NCORES):
        out[assign[i]] = res.results[i]["o"]

    # valid_len == 0: reference softmaxes an all-masked row -> uniform weights.
    for h in np.nonzero(valid_lens == 0)[0]:
        out[h] = values[h].mean(axis=0, keepdims=True)

    return out, res


def kernel(queries, keys, values, valid_lens):
    out, _ = _run(queries, keys, values, valid_lens)
    return out
